# revision 20
# baseline (speedup 1.0000x reference)
"""CQT (constant-Q transform) + amplitude_to_db kernel for Trainium2.

Full-input contract: kernel(x) takes x [32, 64000] f32 and returns
[32, 84, 126] f32, matching:

    frames = pad(x, n_fft//2)[:, t*HOP + n]          # [B, 126, 16384]
    cr/ci  = frames @ Kr.T / Ki.T                    # [B, 84, 126]
    mag    = sqrt(cr^2 + ci^2)
    out    = amplitude_to_db(mag, ref=max per item, amin=1e-5, top_db=80)

Sharding: pure data parallelism - 4 batch items per NeuronCore on 8 cores.

v3 design (hybrid precision, tight schedule):
  * Contraction over n_fft=16384 split into 128-row chunks; 26+4 CENTRAL
    chunks run fp16, 64 TAIL chunks run as 32 fp8e4 DoubleRow pairs
    (2 chunks per PE pass).  Per-bin pow2 scales keep the low-precision
    weights in range; the inverse scales fold into the Ln epilogue.
  * PSUM is initialized by start=True on the first matmul touching each
    bank region (PE executes in program order), so no zeroing passes.
  * DMA priority: w16 slab0 + x16 land first so real fp16 matmuls start
    ~1us after the input DMAs begin; junk warmup matmuls (vector-memset
    scratch) hold the PE p-state ramp only while real data is in flight.
  * Per T-half matmul order B -> A-fp16 -> fp8-DR matches DMA arrival and
    finalizes psB early so its (tiny) squares hide under the A matmuls.
  * Epilogue in ln domain with the amin clamp folded as a +AMIN^2 bias
    (differences only below the -80 dB floor) and the pow2 unscale folded
    as the Ln per-partition scale:  lnm = Ln(m2raw * INVS2 + AMIN^2).
    Squares split ACT (A-re) / DVE (A-im, B); per-item finals alternate
    ACT Relu(lnm + (C - lnr)) and DVE add/max, so the two engines run the
    four items pairwise in parallel.  db stores max(lnm - lnr + C, 0);
    the host applies the exact linear map (db - C) * DB_SCALE.
"""

import os
import numpy as np
import ml_dtypes

import concourse.bass as bass
import concourse.mybir as mybir
from concourse import bacc
from concourse import bass_isa
from concourse.ap import AP
from concourse.bass_utils import run_bass_kernel_spmd

# ---- problem constants (hardcoded; must match the reference) ----
SR = 22050
HOP = 512
N_BINS = 84
BPO = 12
FMIN = 32.70319566257483
AMIN = 1e-5
TOP_DB = 80.0
B = 32
N_SAMP = 64000
N_CORES = 8
NI = B // N_CORES            # items per core = 4
T = 1 + N_SAMP // HOP        # 126 frames
TH = T // 2                  # 63, T-half for epilogue overlap
DB_SCALE = 10.0 / np.log(10.0)  # 20*log10(mag) == DB_SCALE * ln(mag^2)
CLN = float(TOP_DB / DB_SCALE)  # top_db in ln(mag^2) domain = 18.4207
X8_SCALE = 32.0              # x prescale into fp8e4's sweet spot

P = 128
NLOW = 24                    # bins 0-23 live in the fp8 tail chunks
NB_BINS = N_BINS - 64        # 20

N_JUNK = int(os.environ.get("CQT_NJUNK", "4"))
JW = int(os.environ.get("CQT_JW", "512"))      # junk matmul width


def _build_cqt_kernels():
    """Same construction as the reference (nnAudio-style direct CQT bank)."""
    Q = 1.0 / (2.0 ** (1.0 / BPO) - 1.0)
    freqs = FMIN * 2.0 ** (np.arange(N_BINS) / BPO)
    lengths = np.ceil(Q * SR / freqs).astype(int)
    n_fft = int(2 ** np.ceil(np.log2(lengths.max())))
    K = np.zeros((N_BINS, n_fft), dtype=np.complex128)
    for k in range(N_BINS):
        L = int(lengths[k])
        t = np.arange(L) - (L - 1) / 2.0
        kern = np.hanning(L) * np.exp(2j * np.pi * freqs[k] * t / SR)
        kern /= np.abs(kern).sum()
        kern /= np.sqrt(L)
        s = (n_fft - L) // 2
        K[k, s:s + L] = kern
    return K.real.astype(np.float32), K.imag.astype(np.float32), n_fft


Kr, Ki, N_FFT = _build_cqt_kernels()
PAD = N_FFT // 2
FW = (N_SAMP + 2 * PAD) // P      # 628 free-dim width of column-major xp
QW = FW // 4                      # 157
assert (N_SAMP + 2 * PAD) % P == 0 and HOP == 4 * P

# per-bin pow2 scales: fp8 weight tiles use S8, fp16 tiles use S16 = 32*S8 so
# that psum accumulations (fp8 path has x pre-scaled by 32) are consistent.
_mx = np.maximum(np.abs(Kr).max(axis=1), np.abs(Ki).max(axis=1))
S8 = np.exp2(np.floor(np.log2(224.0 / _mx))).astype(np.float32)   # [84]
S16 = S8 * X8_SCALE
INVS2 = (1.0 / (S16 * S16)).astype(np.float32)    # pow2 unscale, exact in f32

# ---- chunk classification ----
# A-group support (bins 0-63) is chunks [19, 109); oct2 (bins 24-35) support
# is [52, 76).  fp16 centrals: {51} u [52,76) u {108} (26 chunks, M=128).
# fp8 tails: [19,51) and [76,108) -> 2x32 chunks -> 32 DoubleRow pairs.
# B-group (bins 64-83): chunks [62, 66), fp16, M=64 (padded to 128).
C16A = [51] + list(range(52, 76)) + [108]
C16B = [62, 63, 64, 65]


def _side_pairs(a, b):
    # pair (c, c+16): same phase, rhs slot step = 16 elements (the DoubleRow
    # ISA requires the Ko-dim step to be a multiple of 16)
    d = (b - a) // 2
    assert d == 16
    return [(c, c + d) for c in range(a, a + d)]


PAIRS = sorted(_side_pairs(19, 51) + _side_pairs(76, 108),
               key=lambda p: (p[0] % 4, p[0]))     # 32 pairs, phase-ordered
NPAIR = len(PAIRS)
M8 = 96          # fp8 stationary width: re 0:24, zeros, im 64:88, pad to 96
W8COLS = 2 * M8  # per pair tile (DoubleRow interleave)

# fp16 centrals in consumption order: phase{2,3} chunks first (their x16
# half is DMA'd concurrently with ph{0,1}), B tiles first within each group
CENT = sorted([("A", c) for c in C16A] + [("B", c) for c in C16B],
              key=lambda kc: (0 if kc[1] % 4 >= 2 else 1,
                              0 if kc[0] == "B" else 1, kc[1]))
LAST_B = max(j for j, (k, _) in enumerate(CENT) if k == "B")
CENT_OFF = np.arange(len(CENT) + 1) * P
W16COLS = int(CENT_OFF[-1])          # 30*128 = 3840

# w16 DMA slabs in consumption order across queues
W16_SLAB = [0, 8, 16, 24, 30]
# w8 DMA slabs (pair index ranges), consumption-ordered across queues
W8_SLAB = [0, 2, 7, 12, 22, 32]

# x16 q-trim: centrals only read q in [12, 153)
XQ0, XQ1 = 12, 153


def _pack_weights():
    KrT, KiT = Kr.T, Ki.T        # [N_FFT, 84]
    w16 = np.zeros((P, W16COLS), np.float32)
    for j, (kind, c) in enumerate(CENT):
        o = int(CENT_OFF[j])
        blk = slice(c * P, (c + 1) * P)
        if kind == "A":
            w16[:, o:o + 64] = KrT[blk, :64] * S16[:64]
            w16[:, o + 64:o + 128] = KiT[blk, :64] * S16[:64]
        else:
            w16[:, o:o + NB_BINS] = KrT[blk, 64:] * S16[64:]
            w16[:, o + 32:o + 32 + NB_BINS] = KiT[blk, 64:] * S16[64:]
            # cols o+52 .. o+128 stay zero (pad to full FWL width)
    w8 = np.zeros((P, NPAIR * W8COLS), np.float32)
    for j, (c0, c1) in enumerate(PAIRS):
        for s, c in enumerate((c0, c1)):
            o = j * W8COLS + s * M8
            blk = slice(c * P, (c + 1) * P)
            w8[:, o:o + NLOW] = KrT[blk, :NLOW] * S8[:NLOW]
            w8[:, o + 64:o + 64 + NLOW] = KiT[blk, :NLOW] * S8[:NLOW]
    return w16.astype(np.float16), w8.astype(ml_dtypes.float8_e4m3)


W16, W8 = _pack_weights()
# col0 = pow2 unscale (Ln scale), col1 = AMIN^2 (Ln bias, folds the clamp)
CONSTS = np.stack([INVS2, np.full(N_BINS, AMIN * AMIN, np.float32)],
                  axis=1)                         # [84, 2] f32


def build_program():
    nc = bacc.Bacc("TRN2", target_bir_lowering=False, debug=False,
                   enable_asserts=True)
    f32 = mybir.dt.float32
    f16 = mybir.dt.float16
    f8 = mybir.dt.float8e4

    x16_in = nc.dram_tensor("x16_in", [P, NI * FW], f16, kind="ExternalInput").ap()
    x8_in = nc.dram_tensor("x8_in", [P, NI * FW], f8, kind="ExternalInput").ap()
    w16_in = nc.dram_tensor("w16_in", [P, W16COLS], f16, kind="ExternalInput").ap()
    w8_in = nc.dram_tensor("w8_in", [P, NPAIR * W8COLS], f8,
                           kind="ExternalInput").ap()
    cst_in = nc.dram_tensor("cst_in", [N_BINS, 2], f32, kind="ExternalInput").ap()
    out = nc.dram_tensor("out", [N_BINS, NI * T], f16, kind="ExternalOutput").ap()

    xt16 = nc.alloc_sbuf_tensor("xt16", [P, NI * FW], f16).ap()
    xt8 = nc.alloc_sbuf_tensor("xt8", [P, NI * FW], f8).ap()
    wt16 = nc.alloc_sbuf_tensor("wt16", [P, W16COLS], f16).ap()
    wt8 = nc.alloc_sbuf_tensor("wt8", [P, NPAIR * W8COLS], f8).ap()
    cst = nc.alloc_sbuf_tensor("cst", [N_BINS, 2], f32).ap()
    junk = nc.alloc_sbuf_tensor("junk", [P, JW], f16).ap()
    tmp = nc.alloc_sbuf_tensor("tmp", [N_BINS, NI * T], f32).ap()
    tmp2 = nc.alloc_sbuf_tensor("tmp2", [64, NI * T], f32).ap()
    m2 = nc.alloc_sbuf_tensor("m2", [N_BINS, NI * T], f32).ap()
    lnm = nc.alloc_sbuf_tensor("lnm", [N_BINS, NI * T], f32).ap()
    db = nc.alloc_sbuf_tensor("db", [N_BINS, NI * T], f16).ap()
    r2 = nc.alloc_sbuf_tensor("r2", [N_BINS, 2 * NI], f32).ap()
    r1 = nc.alloc_sbuf_tensor("r1", [N_BINS, NI], f32).ap()
    rall = nc.alloc_sbuf_tensor("rall", [N_BINS, NI], f32).ap()
    biasv = nc.alloc_sbuf_tensor("biasv", [N_BINS, NI], f32).ap()
    lnwarm = nc.alloc_sbuf_tensor("lnwarm", [1, 4], f32).ap()

    # one PSUM bank pair per T-half so the h0 epilogue can read its banks
    # while the PE is still accumulating the h1 banks
    psA2 = [nc.alloc_psum_tensor(f"psA{h}", [P, NI * TH], f32).ap()
            for h in range(2)]
    psB2 = [nc.alloc_psum_tensor(f"psB{h}", [P, NI * TH], f32).ap()
            for h in range(2)]
    psW = nc.alloc_psum_tensor("psW", [P, JW], f32).ap()

    s_x16 = [nc.alloc_semaphore(f"s_x16{r}") for r in range(4)]
    s_x8 = [nc.alloc_semaphore(f"s_x8{r}") for r in range(4)]
    s_w16 = [nc.alloc_semaphore(f"s_w16{i}") for i in range(4)]
    s_w8 = [nc.alloc_semaphore(f"s_w8{i}") for i in range(5)]
    s_cst = nc.alloc_semaphore("s_cst")
    s_mi = nc.alloc_semaphore("s_mi")
    s_pe = nc.alloc_semaphore("s_pe")     # 1=psB0, 2=psA0, 3=psB1, 4=psA1
    s_a = nc.alloc_semaphore("s_a")       # ACT epilogue steps
    s_v = nc.alloc_semaphore("s_v")       # DVE epilogue steps
    s_g2 = nc.alloc_semaphore("s_g2")     # gpsimd all-reduce done
    s_o1 = nc.alloc_semaphore("s_o1")
    s_o2 = nc.alloc_semaphore("s_o2")

    # x SBUF layout is (r, q, i): col = r*628 + q*4 + i, so (t, i) merges
    # into one contiguous 252-wide moving dim per T-half and psum columns
    # are t-major/item-minor.
    HC = NI * TH                                  # 252 columns per T-half

    Ln = mybir.ActivationFunctionType.Ln
    Square = mybir.ActivationFunctionType.Square
    Relu = mybir.ActivationFunctionType.Relu
    Amax = mybir.AluOpType.max
    Asub = mybir.AluOpType.subtract
    Aadd = mybir.AluOpType.add
    Amult = mybir.AluOpType.mult

    def hslice(ap_pit, h):
        return ap_pit[:, h * HC:(h + 1) * HC]

    def cent_rhs(c, h):
        r, q0 = c % 4, c // 4
        o = r * FW + (q0 + h * TH) * NI
        return xt16[:, o:o + HC]

    def pair_rhs(c, h):
        """DoubleRow rhs [128][2 slots][252] for pair (c, c+16), T-half h."""
        r, q0 = c % 4, c // 4
        o = r * FW + (q0 + h * TH) * NI
        base = xt8[:, o:o + HC]
        ap_l = [list(d) for d in base.ap]
        return AP(base.tensor, base.offset,
                  [ap_l[0], [16, 2], [1, HC]])

    def w8_sem(j):
        for i in range(5):
            if j < W8_SLAB[i + 1]:
                return s_w8[i]
        raise IndexError(j)

    def w16_sem(j):
        for i in range(4):
            if j < W16_SLAB[i + 1]:
                return s_w16[i]
        raise IndexError(j)

    def dma_x16(eng, r, sem):
        o = r * FW + XQ0 * NI
        w = (XQ1 - XQ0) * NI
        eng.dma_start(xt16[:, o:o + w], x16_in[:, o:o + w]).then_inc(sem, 16)

    def dma_x8(eng, r):
        o = r * FW
        eng.dma_start(xt8[:, o:o + NI * QW],
                      x8_in[:, o:o + NI * QW]).then_inc(s_x8[r], 16)

    def dma_w8(eng, i):
        o0, o1 = W8_SLAB[i] * W8COLS, W8_SLAB[i + 1] * W8COLS
        eng.dma_start(wt8[:, o0:o1], w8_in[:, o0:o1]).then_inc(s_w8[i], 16)

    def dma_w16(eng, i):
        t0, t1 = int(CENT_OFF[W16_SLAB[i]]), int(CENT_OFF[W16_SLAB[i + 1]])
        eng.dma_start(wt16[:, t0:t1], w16_in[:, t0:t1]).then_inc(s_w16[i], 16)

    # ---- pre-block: input DMAs + PE warmup start before the block-entry
    # sync, hiding the DMA-ring spin-up latency ----
    nc.vector.memset(junk[:], float(os.environ.get("CQT_JUNKVAL", "0"))
                     ).then_inc(s_mi, 1)
    dma_x8(nc.scalar, 0)
    dma_x16(nc.scalar, 2, s_x16[2])
    dma_w16(nc.scalar, 0)                     # tiles [0,8)
    dma_w16(nc.scalar, 1)                     # tiles [8,16)
    dma_w8(nc.sync, 0)                        # pairs [0,2)
    dma_w8(nc.sync, 1)                        # pairs [2,7)
    dma_w8(nc.sync, 2)                        # pairs [7,12)
    dma_w8(nc.sync, 3)                        # pairs [12,22)
    dma_x16(nc.sync, 3, s_x16[3])
    dma_x16(nc.sync, 1, s_x16[1])
    dma_x8(nc.gpsimd, 1)
    dma_x8(nc.gpsimd, 2)
    dma_x8(nc.gpsimd, 3)
    dma_w8(nc.gpsimd, 4)                      # pairs [22,32)
    dma_x16(nc.gpsimd, 0, s_x16[0])
    dma_w16(nc.gpsimd, 2)                     # tiles [16,24)
    # junk warmup + psum zeroing can also run pre-block (dep: s_mi only)
    nc.tensor.wait_ge(s_mi, 1)
    for _ in range(N_JUNK):
        nc.tensor.matmul(psW[:], lhsT=junk[:, :P], rhs=junk[:, :JW],
                         start=True, stop=True)
    for h in range(2):
        nc.tensor.matmul(psA2[h][:], lhsT=junk[:, :P],
                         rhs=junk[:, :HC], start=True,
                         stop=True, skip_group_check=True)
        nc.tensor.matmul(psB2[h][:], lhsT=junk[:, :P],
                         rhs=junk[:, :HC], start=True,
                         stop=True, skip_group_check=True)

    with nc.Block(no_gpsimd_drain=True) as block:

        @block.scalar
        def _(scalar):
            # preload act tables (Ln / Square / Relu) while DMAs fly
            scalar.activation(lnwarm[:, 0:1], nc.const_aps.tensor(1.0, (1, 1)), Ln)
            scalar.activation(lnwarm[:, 1:2], nc.const_aps.tensor(1.0, (1, 1)),
                              Square)
            scalar.activation(lnwarm[:, 2:3], nc.const_aps.tensor(1.0, (1, 1)),
                              Relu)
            dma_w16(scalar, 3)                 # tiles [24,30)
            scalar.dma_start(cst[:], cst_in).then_inc(s_cst, 16)
            scalar.wait_ge(s_cst, 16)
            for h in range(2):
                # B squares early (psB finalizes mid-centrals); A-re square
                # once psA is final; A-im square runs on the DVE in parallel
                scalar.wait_ge(s_pe, 2 * h + 1)
                scalar.activation(hslice(m2[64:84], h), psB2[h][0:20], Square)
                scalar.activation(hslice(tmp[64:84], h), psB2[h][32:52],
                                  Square)
                scalar.wait_ge(s_pe, 2 * h + 2)
                scalar.activation(hslice(m2[0:64], h), psA2[h][0:64],
                                  Square).then_inc(s_a)          # s_a = 2h+1
                # ln pass: amin clamp folded as +AMIN^2 bias, pow2 unscale
                # folded as per-bin scale; relayouts (t,i) -> (i,t) so the
                # per-item finals read/write contiguously
                scalar.wait_ge(s_v, 2 * h + 1)
                scalar.activation(
                    lnm.rearrange("p (i t) -> p t i",
                                  i=NI)[:, h * TH:(h + 1) * TH],
                    m2.rearrange("p (t i) -> p t i",
                                 i=NI)[:, h * TH:(h + 1) * TH],
                    Ln, bias=cst[:, 1:2],
                    scale=cst[:, 0:1]).then_inc(s_a)             # s_a = 2h+2
            # ref: ln of the all-reduced per-item max (true mag^2 domain)
            scalar.wait_ge(s_g2, 1)
            scalar.activation(rall[:], rall[:], Ln,
                              bias=cst[:, 1:2]).then_inc(s_a)    # s_a = 5
            # finals: items 0 and 2 via Relu(lnm + (C - lnr)); items 1,3 on DVE
            scalar.wait_ge(s_v, 6)
            scalar.activation(db[:, 0:T], lnm[:, 0:T], Relu,
                              bias=biasv[:, 0:1]).then_inc(s_a)  # s_a = 6
            scalar.activation(db[:, 2 * T:3 * T], lnm[:, 2 * T:3 * T], Relu,
                              bias=biasv[:, 2:3])
            scalar.wait_ge(s_v, 8)
            scalar.dma_start(out[:, 2 * T:], db[:, 2 * T:]).then_inc(s_o2, 16)
            scalar.wait_ge(s_o2, 16)

        @block.sync
        def _(sync):
            sync.wait_ge(s_a, 6)
            sync.wait_ge(s_v, 7)
            sync.dma_start(out[:, 0:2 * T], db[:, 0:2 * T]).then_inc(s_o1, 16)
            sync.wait_ge(s_o1, 16)

        @block.gpsimd
        def _(gpsimd):
            gpsimd.wait_ge(s_v, 5)
            gpsimd.partition_all_reduce(rall[:], r1[:], channels=N_BINS,
                                        reduce_op=bass_isa.ReduceOp.max
                                        ).then_inc(s_g2, 1)

        @block.vector
        def _(vector):
            for h in range(2):
                # A-im square on the DVE (psum -> copy -> mult; the DVE can
                # read only one psum operand per op) in parallel with ACT's
                # A-re square
                vector.wait_ge(s_pe, 2 * h + 2)
                vector.tensor_copy(hslice(tmp2, h), psA2[h][64:128])
                vector.tensor_tensor(hslice(tmp[0:64], h), hslice(tmp2, h),
                                     hslice(tmp2, h), Amult)
                vector.wait_ge(s_a, 2 * h + 1)
                vector.tensor_tensor(hslice(m2, h), hslice(m2, h),
                                     hslice(tmp, h), Aadd)
                vector.drain().then_inc(s_v, 1)                  # s_v = 2h+1
                # per-item max over this half's frames on the raw scaled m2
                # (runs concurrently with the ACT ln pass; unscaled below)
                vector.tensor_reduce(
                    r2[:, h * NI:(h + 1) * NI],
                    m2.rearrange("p (t i) -> p i t",
                                 i=NI)[:, :, h * TH:(h + 1) * TH],
                    axis=mybir.AxisListType.X, op=Amax)
                vector.drain().then_inc(s_v, 1)                  # s_v = 2h+2
            vector.tensor_tensor(r1[:], r2[:, 0:NI], r2[:, NI:2 * NI], Amax)
            vector.tensor_scalar_mul(r1[:], r1[:], cst[:, 0:1])
            vector.drain().then_inc(s_v, 1)                      # s_v = 5
            # biasv = C - lnr  (per item, replicated over bins by allreduce)
            vector.wait_ge(s_a, 5)
            vector.tensor_scalar(biasv[:], rall[:], CLN, -1.0, Asub, Amult)
            vector.drain().then_inc(s_v, 1)                      # s_v = 6
            vector.tensor_scalar(db[:, T:2 * T], lnm[:, T:2 * T],
                                 biasv[:, 1:2], 0.0, Aadd, Amax)
            vector.drain().then_inc(s_v, 1)                      # s_v = 7
            vector.tensor_scalar(db[:, 3 * T:], lnm[:, 3 * T:],
                                 biasv[:, 3:4], 0.0, Aadd, Amax)
            vector.drain().then_inc(s_v, 1)                      # s_v = 8

        @block.tensor
        def _(tensor):
            waited = set()

            def need(sem):
                if id(sem) not in waited:
                    tensor.wait_ge(sem, 16)
                    waited.add(id(sem))

            # fp8 DoubleRow tail pairs, phase-ordered (both T-halves)
            for j, (c0, c1) in enumerate(PAIRS):
                need(s_x8[c0 % 4])
                need(w8_sem(j))
                wtile = wt8[:, j * W8COLS:(j + 1) * W8COLS].rearrange(
                    "p (two m) -> p two m", two=2)
                for h in range(2):
                    tensor.matmul(psA2[h][0:M8, :],
                                  lhsT=wtile, rhs=pair_rhs(c0, h),
                                  start=False, stop=False,
                                  perf_mode=mybir.MatmulPerfMode.DoubleRow,
                                  skip_group_check=True)

            # fp16 centrals, T-half split for epilogue overlap; psB finishes
            # early (B tiles lead within phase groups) so its squares hide
            # under the A matmuls
            for h in range(2):
                for j, (kind, c) in enumerate(CENT):
                    need(s_x16[c % 4])
                    need(w16_sem(j))
                    rhs = cent_rhs(c, h)
                    o = int(CENT_OFF[j])
                    if kind == "A":
                        tensor.matmul(psA2[h][:],
                                      lhsT=wt16[:, o:o + P], rhs=rhs,
                                      start=False,
                                      stop=(h == 1 and j == len(CENT) - 1),
                                      skip_group_check=True)
                    else:
                        tensor.matmul(psB2[h][:],
                                      lhsT=wt16[:, o:o + P], rhs=rhs,
                                      start=False, stop=False,
                                      skip_group_check=True)
                    if j == LAST_B:
                        tensor.drain().then_inc(s_pe, 1)   # psB[h] final
                tensor.drain().then_inc(s_pe, 1)           # psA[h] final

    nc.compile()
    return nc


def pack_x(x):
    """x [B, 64000] f32 -> per-core phase-major packs (f16, f8)."""
    xp = np.pad(np.asarray(x, dtype=np.float32), ((0, 0), (PAD, PAD)))
    # x_cm[b, p, r, q] = xp[b, (4q+r)*128+p]; chunk c=4q0+r streams in t
    x_cm = xp.reshape(B, QW, 4, P).transpose(0, 3, 2, 1)   # [B,128,4,157]
    packs = []
    for core in range(N_CORES):
        blk = x_cm[core * NI:(core + 1) * NI]              # [NI,128,4,157]
        lay = np.ascontiguousarray(
            blk.transpose(1, 2, 3, 0).reshape(P, NI * FW))  # [p, (r q i)]
        p16 = lay.astype(np.float16)
        p8 = np.clip(lay * X8_SCALE, -240.0, 240.0).astype(ml_dtypes.float8_e4m3)
        packs.append((p16, p8))
    return packs


_PROGRAM = None


def _get_program():
    global _PROGRAM
    if _PROGRAM is None:
        _PROGRAM = build_program()
    return _PROGRAM


def run(x, **spmd_kwargs):
    """Run on 8 NeuronCores; returns (output [32,84,126] f32, results)."""
    nc = _get_program()
    packs = pack_x(x)
    in_maps = [{"x16_in": packs[i][0], "x8_in": packs[i][1],
                "w16_in": W16, "w8_in": W8, "cst_in": CONSTS}
               for i in range(N_CORES)]
    res = run_bass_kernel_spmd(nc, in_maps, core_ids=list(range(N_CORES)),
                               **spmd_kwargs)
    outs = []
    for i in range(N_CORES):
        o = res.results[i]["out"].astype(np.float32)        # [84, (i t)]
        outs.append(o.reshape(N_BINS, NI, T).transpose(1, 0, 2))
    # device stored max(lnm - lnr + C, 0); undo the +C shift and convert
    # ln(mag^2) -> dB (exact linear map)
    full = (np.concatenate(outs, axis=0) - np.float32(CLN)) * np.float32(DB_SCALE)
    return np.ascontiguousarray(full.astype(np.float32)), res


def kernel(x):
    return run(x)[0]
